# revision 18
# baseline (speedup 1.0000x reference)
"""Trainium2 Bass kernel for nn_LASCC (sparse patch-correlation attention + top-k).

Math (per batch element b):
  x_hat = L2-normalize(x, dim=channels)
  z_p[c, n] = x_hat at the two in-patch diagonal pixels (p=0: (0,0), p=1: (1,1))
  C_p = z_p^T z_p                  (1024x1024 normalized correlation, symmetric)
  s_q = alpha * mask * C_q,  C_2 = (C_0 + C_1)/2
  A_q = softmax_row(s) * softmax_col(s) = E^2 / (r[n] r[m]),  E=exp(s), r=rowsum(E)
  out pixel (row,col) with patch n, map q: top-3 over m of A_q[n, m]

Log-space formulation per core (2 batch elements, 3 maps each):
  sm_q  = (2*alpha*mask) .* C_q  = 2*s_q      DVE STT straight out of PSUM
  r     = exp(0.5*sm) rowsums                 ACT exp + accumulator
  W     = sm - lnr[m]_bcast  (in place!)      DMA accumulate (gp software DGE),
                                              broadcast row read from DRAM
  t8    = top-8 of W per row                  DVE max8
  out   = exp(t8[:, :3] - lnr[n])             tiny DVE add + ACT exp
  The avg map (q=2) never touches PE/DVE for its matrix:
  sm_2 = (sm_0 + sm_1)/2 up to the exp scale; built by DMA copy + DMA accumulate.
"""
import numpy as np

import concourse.bass as bass
import concourse.mybir as mybir
from concourse import bacc
from concourse.tile import TileContext
from concourse.bass_utils import run_bass_kernel_spmd

F32 = mybir.dt.float32
FP16 = mybir.dt.float16
AF = mybir.ActivationFunctionType
ALU = mybir.AluOpType

B_FULL = 16
N_CORES = 8
B_LOC = B_FULL // N_CORES  # 2
C = 128
H = W = 64
NPH = 32
NP = 1024  # patches
PS = 2
TOPK = 3
NCHUNK = NP // 128  # 8

LAST_EXEC_NS = None


def _build_mask() -> np.ndarray:
    """(1 - gaussian) self-suppression mask, [NP, NP] float32 (matches reference)."""
    rat_s = np.float32(0.05)
    sr = np.float32(NPH) * rat_s
    ind_r = np.arange(NPH, dtype=np.float32).reshape(1, NPH, 1)
    ind_c = np.arange(NPH, dtype=np.float32).reshape(1, 1, NPH)
    cent = np.arange(NPH, dtype=np.float32)
    cent_r = np.repeat(cent, NPH).reshape(NP, 1, 1)
    cent_c = np.tile(cent, NPH).reshape(NP, 1, 1)
    g = np.exp(-((ind_r - cent_r) ** 2) / (2.0 * sr * sr)) * np.exp(
        -((ind_c - cent_c) ** 2) / (2.0 * sr * sr)
    )
    return (1.0 - g).reshape(NP, NP).astype(np.float32)


def build_nc():
    nc = bacc.Bacc(trn_type="TRN2")

    x_d = nc.dram_tensor("x", [B_LOC, C, H * W], F32, kind="ExternalInput")
    mask_d = nc.dram_tensor("mask", [NP, NP], FP16, kind="ExternalInput")
    out_d = nc.dram_tensor("out", [B_LOC, 3, NP, TOPK], F32, kind="ExternalOutput")

    with TileContext(nc) as tc:
        with tc.tile_pool(name="const", bufs=1) as cpool, \
             tc.tile_pool(name="slab", bufs=2) as slabp, \
             tc.tile_pool(name="work", bufs=3) as work, \
             tc.tile_pool(name="small", bufs=3) as small, \
             tc.tile_pool(name="ps", bufs=2, space="PSUM") as ps, \
             tc.tile_pool(name="dsc", bufs=3, space="DRAM") as dsc:

            # ---- constants
            mask_sb = cpool.tile([128, NCHUNK, NP], FP16)  # (2a*mask)[128i+p, m]
            nc.sync.dma_start(
                mask_sb, mask_d[:, :].rearrange("(i p) m -> p i m", p=128)
            )
            ones_k = cpool.tile([128, 1], FP16)  # norm column-sum matmul lhsT
            nc.vector.memset(ones_k, 1.0)

            # ---- phase N: norms + normalized z (fp16), 4 chains (b, p)
            chains = []
            for b in range(B_LOC):
                for p in range(PS):
                    xs = work.tile([128, NPH, W], F32, name=f"xs{b}{p}",
                                   tag="xs", bufs=4)
                    # rows p::2 of the image
                    nc.sync.dma_start(
                        xs, x_d[b].rearrange("c (i r j) -> c r i j",
                                             r=PS, j=W)[:, p])
                    # elements x[c, 2i+p, 2j+p] as [c, 1024]
                    zv = xs.rearrange("c i (j s) -> c s (i j)", s=PS)[:, p]
                    chains.append((b, p, zv))

            zsqs = {}
            for b, p, zv in chains:
                zsq = work.tile([128, NP], FP16, name="zsq", tag="zsq", bufs=2)
                nc.vector.tensor_tensor(out=zsq, in0=zv, in1=zv, op=ALU.mult)
                zsqs[(b, p)] = zsq
            nrms = {}
            for b, p, zv in chains:
                nrm = ps.tile([1, NP], F32, name="nrm", tag="rrow")
                for h in range(2):
                    nc.tensor.matmul(nrm[:, 512 * h:512 * (h + 1)], ones_k,
                                     zsqs[(b, p)][:, 512 * h:512 * (h + 1)],
                                     start=True, stop=True)
                nrms[(b, p)] = nrm
            lnns = {}
            for b, p, zv in chains:  # all Lns batched: one table switch
                lnn = small.tile([1, NP], F32, name="lnn", tag="lnn", bufs=4)
                nc.scalar.activation(lnn, nrms[(b, p)], AF.Ln)
                lnns[(b, p)] = lnn
            invs = {}
            for b, p, zv in chains:  # then all Exps
                inv1 = small.tile([1, NP], F32, name="inv1", tag="inv1", bufs=2)
                nc.scalar.activation(inv1, lnns[(b, p)], AF.Exp, scale=-0.5)
                i_dram = dsc.tile([NP], F32, name="i_dram", tag="i_dram",
                                  bufs=2)
                nc.sync.dma_start(
                    i_dram[:].rearrange("(a m) -> a m", a=1), inv1)
                invs[(b, p)] = i_dram
            zp = {}
            for b, p, zv in chains:
                ibc = work.tile([128, NP], F32, name="ibc", tag="ibc", bufs=2)
                nc.sync.dma_start(
                    ibc, invs[(b, p)][:].rearrange(
                        "(a m) -> a m", a=1).broadcast_to([128, NP]))
                z = cpool.tile([128, NP], FP16, name=f"z{b}{p}", tag=f"z{b}{p}",
                               bufs=1)
                nc.vector.tensor_tensor(out=z, in0=zv, in1=ibc, op=ALU.mult)
                zp[(b, p)] = z

            # ---- phase M: six (b, q) stages, chunk-interleaved pipeline.
            SLABTAG = {0: "slab0", 1: "slab1", 2: "slabt"}

            def open_E(b, q):
                if q == 2:
                    slab = tmp_slabs[b]
                else:
                    slab = slabp.tile([128, NCHUNK, NP], FP16, name=f"sl{q}",
                                      tag=SLABTAG[q], bufs=2)
                rT = small.tile([128, NCHUNK], F32, name="rT", tag="rT",
                                bufs=2)
                stg = dict(slab=slab, rT=rT, b=b, q=q)
                if q == 0:
                    # tmp slab for the avg map; F(b,0) copies pristine sm0 in
                    stg["tmp"] = slabp.tile([128, NCHUNK, NP], FP16,
                                            name="slt", tag=SLABTAG[2], bufs=2)
                    tmp_slabs[b] = stg["tmp"]
                return stg

            def emit_E_chunk(stg, i):
                b, q, slab = stg["b"], stg["q"], stg["slab"]
                if q < 2:
                    zq = zp[(b, q)]
                    G = ps.tile([128, NP], F32, name="G", tag="G")
                    for h in range(2):
                        nc.tensor.matmul(G[:, 512 * h:512 * (h + 1)],
                                         zq[:, 128 * i:128 * (i + 1)],
                                         zq[:, 512 * h:512 * (h + 1)],
                                         start=True, stop=True)
                    nc.vector.scalar_tensor_tensor(
                        out=slab[:, i, :], in0=G, scalar=1.0,
                        in1=mask_sb[:, i, :], op0=ALU.mult, op1=ALU.mult)
                    scale = 0.5
                else:
                    # tmp (= pristine sm0) += sm1 -> sm0+sm1 = 4*s_2
                    if i < 5:
                        nc.vector.tensor_tensor(out=slab[:, i, :],
                                                in0=slab[:, i, :],
                                                in1=st[(b, 1)][:, i, :],
                                                op=ALU.add)
                    else:
                        nc.gpsimd.dma_start(slab[:, i, :],
                                            st[(b, 1)][:, i, :],
                                            accum_op=ALU.add)
                    scale = 0.25
                e_scr = work.tile([128, NP], FP16, name="e_scr", tag="e_scr",
                                  bufs=2)
                nc.scalar.activation(e_scr, slab[:, i, :], AF.Exp, scale=scale,
                                     accum_out=stg["rT"][:, i:i + 1])

            def emit_E_tail(stg):
                # nlr = -ln(rT) without the LN table: bit-trick estimate
                # x0 = bits(r)*s1 + s2, one Newton step x1 = x0 + r*e^-x0 - 1
                # (keeps the ACT engine exp-table-resident; err ~5e-4)
                q, rT = stg["q"], stg["rT"]
                x0 = small.tile([128, NCHUNK], F32, name="x0", tag="x0",
                                bufs=2)
                nc.vector.tensor_scalar(
                    out=x0, in0=rT[:, :].bitcast(mybir.dt.int32),
                    scalar1=8.262958405176314e-08, scalar2=-87.98997108849321,
                    op0=ALU.mult, op1=ALU.add)
                e0 = small.tile([128, NCHUNK], F32, name="e0", tag="e0",
                                bufs=2)
                nc.scalar.activation(e0, x0, AF.Exp, scale=-1.0)
                t0 = small.tile([128, NCHUNK], F32, name="t0", tag="t0",
                                bufs=2)
                nc.vector.tensor_tensor(out=t0, in0=rT, in1=e0, op=ALU.mult)
                x1 = small.tile([128, NCHUNK], F32, name="x1", tag="x1",
                                bufs=2)
                nc.vector.scalar_tensor_tensor(
                    out=x1, in0=t0, scalar=-1.0, in1=x0,
                    op0=ALU.add, op1=ALU.add)
                nlrT = small.tile([128, NCHUNK], FP16, name="nlrT", tag="nlrT",
                                  bufs=2)
                nc.vector.tensor_scalar_mul(nlrT, x1,
                                            -1.0 if q < 2 else -2.0)
                u_dram = dsc.tile([NP], FP16, name="u_dram", tag="u_dram")
                nc.sync.dma_start(
                    u_dram[:].rearrange("(i p) -> p i", p=128), nlrT)
                stg["nlrT"] = nlrT
                stg["u_dram"] = u_dram

            def open_F(stg):
                stg["t8c"] = small.tile([128, NCHUNK, 8], FP16, name="t8c",
                                        tag="t8c", bufs=2)
                # chunks whose W-add runs on DVE (q1's F lands in the
                # engine-light avg-map window); the rest go per-chunk through
                # gpsimd accumulate-DMAs
                stg["w_dve"] = range(4) if stg["q"] == 1 else range(0)
                nlr_bc = work.tile([128, NP], FP16, name="nlr_bc",
                                   tag="nlr_bc", bufs=2)
                nc.sync.dma_start(
                    nlr_bc, stg["u_dram"][:].rearrange(
                        "(a m) -> a m", a=1).broadcast_to([128, NP]))
                stg["nlr_bc"] = nlr_bc

            def emit_F_chunk(stg, i):
                slab = stg["slab"]
                if stg["q"] == 0:
                    # stash pristine sm0 for the avg map before W clobbers it
                    nc.sync.dma_start(stg["tmp"][:, i, :], slab[:, i, :])
                # W = sm + (-lnr[m]) broadcast, in place
                if i in stg["w_dve"]:
                    nc.vector.tensor_tensor(out=slab[:, i, :],
                                            in0=slab[:, i, :],
                                            in1=stg["nlr_bc"], op=ALU.add)
                else:
                    nc.gpsimd.dma_start(
                        slab[:, i, :],
                        stg["u_dram"][:].rearrange(
                            "(a m) -> a m", a=1).broadcast_to([128, NP]),
                        accum_op=ALU.add)
                nc.vector.max(out=stg["t8c"][:, i, :],
                              in_=stg["slab"][:, i, :])

            def emit_F_final(stg):
                b, q = stg["b"], stg["q"]
                y = small.tile([128, NCHUNK, TOPK], FP16, name="y", tag="y",
                               bufs=2)
                nc.vector.tensor_tensor(
                    out=y, in0=stg["t8c"][:, :, :TOPK],
                    in1=stg["nlrT"][:, :, None].broadcast_to(
                        [128, NCHUNK, TOPK]),
                    op=ALU.add)
                oacc = small.tile([128, NCHUNK, TOPK], F32, name="oacc",
                                  tag="oacc", bufs=2)
                nc.scalar.activation(oacc, y, AF.Exp,
                                     scale=1.0 if q < 2 else 0.5)
                dst = out_d[b, q].rearrange("(i p) k -> p i k", p=128)
                nc.sync.dma_start(dst, oacc)

            st = {}
            tmp_slabs = {}
            stages = [(b, q) for b in range(B_LOC) for q in range(3)]
            prev = None
            for (b, q) in stages:
                # two-block emission per window: all E-chunks of stage k+1
                # first (immediately runnable), then all F-chunks of stage k
                # (gated on the stage-k normalizer roundtrip) — keeps the
                # in-order engine queues free of head-of-line blocking.
                cur = open_E(b, q)
                if q < 2:
                    st[(b, q)] = cur["slab"]
                for i in range(NCHUNK):
                    emit_E_chunk(cur, i)
                if prev is not None:
                    open_F(prev)
                    for i in range(NCHUNK):
                        emit_F_chunk(prev, i)
                emit_E_tail(cur)
                if prev is not None:
                    emit_F_final(prev)
                prev = cur
            open_F(prev)
            for i in range(NCHUNK):
                emit_F_chunk(prev, i)
            emit_F_final(prev)

    nc.compile()
    return nc


_NC_CACHE = None


def _get_nc():
    global _NC_CACHE
    if _NC_CACHE is None:
        _NC_CACHE = build_nc()
    return _NC_CACHE


def kernel(x: np.ndarray, alpha: np.ndarray) -> np.ndarray:
    global LAST_EXEC_NS
    x = np.ascontiguousarray(np.asarray(x, dtype=np.float32))
    a = float(np.asarray(alpha))
    mask2a = (2.0 * a * _build_mask()).astype(np.float16)

    nc = _get_nc()
    in_maps = []
    for core in range(N_CORES):
        xs = x[core * B_LOC:(core + 1) * B_LOC].reshape(B_LOC, C, H * W)
        in_maps.append({"x": np.ascontiguousarray(xs), "mask": mask2a})
    res = run_bass_kernel_spmd(nc, in_maps, core_ids=list(range(N_CORES)))
    LAST_EXEC_NS = res.exec_time_ns

    # assemble: out[bg, k, 2i+dr, 2j+dc] from T_q[b, n=i*32+j, k]
    out = np.empty((B_FULL, TOPK, H, W), dtype=np.float32)
    for core in range(N_CORES):
        t = res.results[core]["out"]  # [B_LOC, 3, NP, TOPK]
        for bl in range(B_LOC):
            bg = core * B_LOC + bl
            tq = t[bl].reshape(3, NPH, NPH, TOPK).transpose(0, 3, 1, 2)
            out[bg, :, 0::2, 0::2] = tq[0]
            out[bg, :, 1::2, 1::2] = tq[1]
            out[bg, :, 0::2, 1::2] = tq[2]
            out[bg, :, 1::2, 0::2] = tq[2]
    return out
